# revision 5
# baseline (speedup 1.0000x reference)
"""Trainium2 Bass kernel for the LogicLayer (difflogic) problem, v11.

out[i, o] = c0[o] + ca[o]*a + cb[o]*b + cab[o]*a*b
  with a = x[i, idx_a[o]], b = x[i, idx_b[o]],
  [c0, ca, cb, cab] = softmax(weights[o]) @ GATE_COEFFS.

v4 strategy (8 cores, OUTPUT-sharded, fp16 a / uint8 b / uint8 y):
  - gather a-rows from fp16 xT (8 MiB), b-rows from uint8 xbT (4 MiB),
    write y as uint8 (4 MiB): 16 MiB HBM per core/rep (~44us floor).
  - DVE: t = a*(cab/255) + (cb/255)   (tensor_scalar, 4x)
         u = t*B8                      (tensor_tensor, 1x, u8 operand)
    so u = (a*cab + cb)*b exactly.
  - PE (TensorE): psum[h] = diag(ca_g) @ a  (+)  I @ u   accumulated fp32,
    per half-chunk (2048 batch = 4 psum banks), double-buffered halves.
  - ACT: y8 = Identity(253*psum + (253*c0_g + 1.25)) -> uint8, per half.
  - host decodes y = (y8 - 1.0)/253.
  v5: t-pass runs on ACT (Identity, scale=cab', bias=cb') for 3 of 8
  chunks to balance DVE (~40us) vs ACT (~40us); gather pipeline DEPTH=4.
"""

import numpy as np

BATCH, IN_DIM, OUT_DIM = 4096, 8192, 8192
N_CORES = 8
OSHARD = OUT_DIM // N_CORES     # 1024 outputs per core
P = 128
G = OSHARD // P                 # 8 chunks of 128 outputs
DEPTH = 5                       # gather/out tile pipeline depth
HB = BATCH // 2                 # half-chunk batch width (4 psum banks)
NS = HB // 512                  # 512-wide matmul subtiles per half (4)
ACT_T = (0, 3, 6)               # chunk g values whose t-pass runs on ACT

YSCALE = 253.0
YBIAS_ENC = 1.25
YBIAS_DEC = 1.0

GATE_COEFFS = np.array([
    [0, 0, 0, 0], [0, 0, 0, 1], [0, 1, 0, -1], [0, 1, 0, 0],
    [0, 0, 1, -1], [0, 0, 1, 0], [0, 1, 1, -2], [0, 1, 1, -1],
    [1, -1, -1, 1], [1, -1, -1, 2], [1, 0, -1, 0], [1, 0, -1, 1],
    [1, -1, 0, 0], [1, -1, 0, 1], [1, 0, 0, -1], [1, 0, 0, 0],
], dtype=np.float32)  # [16, 4]

_CACHE = {}

# within-chunk interleave: position j <- sorted position (j%16)*8 + j//16,
# so wrap-channel c (descriptors j%16==c, one per SDMA engine) carries 8
# CONSECUTIVE sorted rows instead of stride-16 samples.
_SIGMA = np.array([(j % 16) * 8 + j // 16 for j in range(128)])


def _perm(ia_c):
    """Core's output permutation: global sort by a-row, then per-128-chunk
    channel interleave for per-engine sequential gather streams."""
    p = np.argsort(ia_c, kind="stable")
    return p.reshape(-1, P)[:, _SIGMA].reshape(-1)


def _build_nc(n_reps=1):
    import concourse.bacc as bacc
    import concourse.bass as bass
    import concourse.mybir as mybir
    from concourse.library_config import mlp
    from contextlib import ExitStack

    f16 = mybir.dt.float16
    f32 = mybir.dt.float32
    i16 = mybir.dt.int16
    u8 = mybir.dt.uint8
    Alu = mybir.AluOpType
    Act = mybir.ActivationFunctionType

    nc = bacc.Bacc("TRN2", target_bir_lowering=False, debug=False,
                   num_devices=N_CORES)
    xt = nc.dram_tensor("xt", [IN_DIM, BATCH], f16, kind="ExternalInput")
    xb = nc.dram_tensor("xb", [IN_DIM, BATCH], u8, kind="ExternalInput")
    idxw = nc.dram_tensor("idxw", [P, 2 * G * 8], i16, kind="ExternalInput")
    cfd = nc.dram_tensor("cfd", [P, G, 2], f32, kind="ExternalInput")
    cfy = nc.dram_tensor("cfy", [P, G, 2], f32, kind="ExternalInput")
    wca = nc.dram_tensor("wca", [P, G, P], f16, kind="ExternalInput")
    wid = nc.dram_tensor("wid", [P, P], f16, kind="ExternalInput")
    y = nc.dram_tensor("y", [P, G, BATCH], u8, kind="ExternalOutput")

    T = n_reps * G

    with ExitStack() as stack:
        ent = stack.enter_context
        idx_sb = ent(nc.sbuf_tensor("idx_sb", [P, 2 * G * 8], i16))
        cfd_sb = ent(nc.sbuf_tensor("cfd_sb", [P, G, 2], f32))
        cfy_sb = ent(nc.sbuf_tensor("cfy_sb", [P, G, 2], f32))
        wca_sb = ent(nc.sbuf_tensor("wca_sb", [P, G, P], f16))
        wid_sb = ent(nc.sbuf_tensor("wid_sb", [P, P], f16))
        ga = ent(nc.sbuf_tensor("ga", [P, DEPTH, 1, BATCH], f16))
        gb8 = ent(nc.sbuf_tensor("gb8", [P, DEPTH, 1, BATCH], u8))
        tt = ent(nc.sbuf_tensor("tt", [P, BATCH], f16))
        tt_a = ent(nc.sbuf_tensor("tt_a", [P, 2, BATCH], f16))
        uu = ent(nc.sbuf_tensor("uu", [P, 3, BATCH], f16))
        y8 = ent(nc.sbuf_tensor("y8", [P, DEPTH, BATCH], u8))
        ps = ent(nc.psum_tensor("ps", [P, 2, HB], f32))
        setup_sem = ent(nc.semaphore("setup"))
        ga_sems = [ent(nc.semaphore(f"gasem{i}")) for i in range(DEPTH)]
        gb_sems = [ent(nc.semaphore(f"gbsem{i}")) for i in range(DEPTH)]
        y_sems = [ent(nc.semaphore(f"ysem{i}")) for i in range(DEPTH)]
        act_sem = ent(nc.semaphore("actsem"))
        dve_sem = ent(nc.semaphore("dvesem"))
        pe_sem = ent(nc.semaphore("pesem"))
        att_sem = ent(nc.semaphore("attsem"))
        block = ent(nc.Block())

        def n_act(t):  # number of ACT-t chunks with index <= t
            return sum(1 for j in range(t + 1) if (j % G) in ACT_T)

        def prev_act(t):  # previous ACT-t chunk index before t, or None
            for j in range(t - 1, -1, -1):
                if (j % G) in ACT_T:
                    return j
            return None

        @block.gpsimd
        def _(gp: bass.BassGpSimd):
            gp.load_library(mlp)
            gp.dma_start(idx_sb[:], idxw[:]).then_inc(setup_sem, 16)
            gp.dma_start(cfd_sb[:], cfd[:]).then_inc(setup_sem, 16)
            gp.dma_start(cfy_sb[:], cfy[:]).then_inc(setup_sem, 16)
            gp.dma_start(wca_sb[:], wca[:]).then_inc(setup_sem, 16)
            gp.dma_start(wid_sb[:], wid[:]).then_inc(setup_sem, 16)
            gp.wait_ge(setup_sem, 80)
            for t in range(T):
                g = t % G
                slot = t % DEPTH
                k = t // DEPTH
                if t >= DEPTH:
                    j = t - DEPTH
                    gp.wait_ge(dve_sem, j + 1)       # DVE consumed a,b8
                    gp.wait_ge(pe_sem, 2 * j + 1)    # PE phase-1 read a
                    gp.wait_ge(ga_sems[slot], 16 * k)
                    gp.wait_ge(gb_sems[slot], 16 * k)
                gp.dma_gather(
                    ga[:, slot, :, :], xt[:, :], idx_sb[:, g * 8:(g + 1) * 8],
                    P, P, BATCH,
                ).then_inc(ga_sems[slot], 16)
                gp.dma_gather(
                    gb8[:, slot, :, :], xb[:, :],
                    idx_sb[:, G * 8 + g * 8: G * 8 + (g + 1) * 8],
                    P, P, BATCH,
                ).then_inc(gb_sems[slot], 16)

        @block.vector
        def _(v: bass.BassVectorEngine):
            v.wait_ge(setup_sem, 80)
            for t in range(T):
                g = t % G
                slot = t % DEPTH
                k = t // DEPTH
                on_act = g in ACT_T
                if not on_act:
                    v.wait_ge(ga_sems[slot], 16 * (k + 1))
                    # t = a*(cab/255) + (cb/255)
                    v.tensor_scalar(
                        tt[:], ga[:, slot, 0, :],
                        cfd_sb[:, g, 0:1], cfd_sb[:, g, 1:2],
                        Alu.mult, Alu.add,
                    )
                v.wait_ge(gb_sems[slot], 16 * (k + 1))
                if t >= 3:
                    v.wait_ge(pe_sem, 2 * t - 4)  # uu slot WAR vs PE phase-2
                if on_act:
                    v.wait_ge(att_sem, n_act(t))
                    src_t = tt_a[:, n_act(t) % 2, :]
                else:
                    src_t = tt[:]
                # u = t*B8  (= (a*cab + cb)*b)
                v.tensor_tensor(
                    uu[:, t % 3, :], src_t, gb8[:, slot, 0, :], Alu.mult,
                ).then_inc(dve_sem, 1)

        @block.tensor
        def _(te: bass.BassTensorEngine):
            te.wait_ge(setup_sem, 80)
            for t in range(T):
                g = t % G
                slot = t % DEPTH
                k = t // DEPTH
                te.wait_ge(ga_sems[slot], 16 * (k + 1))
                # phase 1: ps[h] = diag(ca_g) @ a[h]   (resets psum)
                for h in range(2):
                    if t >= 1:
                        te.wait_ge(act_sem, 2 * (t - 1) + h + 1)  # psum WAR
                    for s in range(NS):
                        lo = h * HB + s * 512
                        te.matmul(
                            ps[:, h, s * 512:(s + 1) * 512],
                            wca_sb[:, g, :],
                            ga[:, slot, 0, lo:lo + 512],
                            start=True, stop=False,
                            skip_group_check=True,
                        )
                # phase 2: ps[h] += I @ u[h]
                te.wait_ge(dve_sem, t + 1)
                for h in range(2):
                    for s in range(NS):
                        lo = h * HB + s * 512
                        mm = te.matmul(
                            ps[:, h, s * 512:(s + 1) * 512],
                            wid_sb[:, :],
                            uu[:, t % 3, lo:lo + 512],
                            start=False, stop=True,
                            skip_group_check=True,
                        )
                        if s == NS - 1:
                            mm.then_inc(pe_sem, 1)

        @block.scalar
        def _(act: bass.BassScalarEngine):
            act.wait_ge(setup_sem, 80)

            def act_tpass(j):
                jg = j % G
                jslot = j % DEPTH
                jk = j // DEPTH
                pa = prev_act(j)
                ppa = prev_act(pa) if pa is not None else None
                if ppa is not None:
                    act.wait_ge(dve_sem, ppa + 1)  # tt_a slot WAR (2 acts back)
                act.wait_ge(ga_sems[jslot], 16 * (jk + 1))
                act.activation(
                    tt_a[:, n_act(j) % 2, :], ga[:, jslot, 0, :], Act.Identity,
                    bias=cfd_sb[:, jg, 1:2], scale=cfd_sb[:, jg, 0:1],
                ).then_inc(att_sem, 1)

            # prologue: t-passes for chunks 0,1 (not covered by lead=t+2)
            for j in (0, 1):
                if j < T and (j % G) in ACT_T:
                    act_tpass(j)
            for t in range(T):
                g = t % G
                slot = t % DEPTH
                lead = t + 2
                if lead < T and (lead % G) in ACT_T:
                    act_tpass(lead)  # run t-pass 2 chunks ahead of the casts
                for h in range(2):
                    act.wait_ge(pe_sem, 2 * t + h + 1)
                    if h == 0 and t >= DEPTH:
                        act.wait_ge(y_sems[slot], 16 * (t // DEPTH))
                    act.activation(
                        y8[:, slot, h * HB:(h + 1) * HB], ps[:, h, :],
                        Act.Identity,
                        bias=cfy_sb[:, g, 1:2], scale=cfy_sb[:, g, 0:1],
                    ).then_inc(act_sem, 1)

        @block.sync
        def _(sp: bass.BassEngine):
            for t in range(T):
                g = t % G
                slot = t % DEPTH
                sp.wait_ge(act_sem, 2 * t + 2)
                sp.dma_start(y[:, g, :], y8[:, slot, :]).then_inc(
                    y_sems[slot], 16)
            for s_i in range(DEPTH):
                sp.wait_ge(y_sems[s_i], 16 * ((T - 1 - s_i) // DEPTH + 1))

    nc.compile()
    return nc


def _prep_host(x, weights, idx_a, idx_b):
    x = np.asarray(x, dtype=np.float32)
    w = np.asarray(weights, dtype=np.float32)
    e = np.exp(w - w.max(axis=1, keepdims=True))
    sm = e / e.sum(axis=1, keepdims=True)
    coeffs = (sm @ GATE_COEFFS).astype(np.float32)          # [8192, 4]
    c0, ca, cb, cab = (coeffs[:, i] for i in range(4))
    xt = np.ascontiguousarray(x.T).astype(np.float16)       # [8192, 4096]
    xb = np.ascontiguousarray(np.round(x.T * 255.0)).astype(np.uint8)
    ia = np.asarray(idx_a).astype(np.int16)
    ib = np.asarray(idx_b).astype(np.int16)

    def wrap(seq):  # [n] -> [128, n/16]: j at [j%16, j//16], tiled to 128
        m = seq.reshape(len(seq) // 16, 16).T
        return np.tile(m, (P // 16, 1))

    in_maps = []
    for c in range(N_CORES):
        sl = slice(c * OSHARD, (c + 1) * OSHARD)
        # sort this core's outputs by a-row index: the 8 KiB a-gather
        # descriptors then walk HBM in ascending row order (page locality).
        # The host un-permutes columns at decode (see kernel()).
        perm = _perm(ia[sl])
        iap = ia[sl][perm]
        ibp = ib[sl][perm]
        c0p_, cap_, cbp_, cabp_ = (v[sl][perm] for v in (c0, ca, cb, cab))
        idxw = np.ascontiguousarray(
            np.concatenate([wrap(iap), wrap(ibp)], axis=1))        # [128,128]
        # per-chunk per-partition coeff scalars: [p, g] <- output g*128+p
        cabp = cabp_.reshape(G, P).T / 255.0
        cbp = cbp_.reshape(G, P).T / 255.0
        cfdc = np.ascontiguousarray(
            np.stack([cabp, cbp], axis=2)).astype(np.float32)      # [128,G,2]
        c0p = c0p_.reshape(G, P).T
        cfyc = np.ascontiguousarray(np.stack(
            [np.full((P, G), YSCALE, np.float32),
             YSCALE * c0p + YBIAS_ENC], axis=2)).astype(np.float32)
        cap = cap_.reshape(G, P).T                                 # [128, G]
        wcac = np.zeros((P, G, P), np.float16)
        for g in range(G):
            wcac[np.arange(P), g, np.arange(P)] = cap[:, g].astype(np.float16)
        widc = np.eye(P, dtype=np.float16)
        in_maps.append({
            "xt": xt, "xb": xb, "idxw": idxw, "cfd": cfdc, "cfy": cfyc,
            "wca": np.ascontiguousarray(wcac), "wid": widc,
        })
    return in_maps


def _in_maps(x, weights, idx_a, idx_b):
    return _prep_host(x, weights, idx_a, idx_b)


def kernel(x, weights, idx_a, idx_b):
    from concourse.bass_utils import run_bass_kernel_spmd

    in_maps = _in_maps(x, weights, idx_a, idx_b)
    if "nc" not in _CACHE:
        _CACHE["nc"] = _build_nc()
    nc = _CACHE["nc"]
    res = run_bass_kernel_spmd(nc, in_maps, list(range(N_CORES)))
    ia_all = np.asarray(idx_a)
    outs = []
    for c in range(N_CORES):
        yc = res.results[c]["y"]                    # [128, G, 4096] uint8
        yf = (yc.astype(np.float32) - YBIAS_DEC) / YSCALE
        yd = yf.transpose(2, 1, 0).reshape(BATCH, OSHARD)
        # device column d holds original output perm[d]; invert the sort
        perm = _perm(ia_all[c * OSHARD:(c + 1) * OSHARD])
        outs.append(yd[:, np.argsort(perm)])
    return np.concatenate(outs, axis=1).astype(np.float32)
